# revision 10
# baseline (speedup 1.0000x reference)
"""Trainium2 Bass kernel: 16-head MHA (S=2048, D=1024, Dk=Dv=64) on 8 NeuronCores.

Sharding: tensor-parallel over heads — 2 heads per core. Each core projects
Q/K/V for its 2 heads, computes scores transposed S^T[t, s] = K_h Q_h^T,
applies exp on the Scalar engine (scale 1/8 fused), and accumulates
heads^T = V_aug^T @ exp(S^T) with a ones-column in V so the softmax
denominator falls out of the same matmul (PSUM row 64).

Engine discipline (v2):
  - et streams in s-chunk-major; Q/K/V matmuls start as each 512-col chunk
    lands, so the PE never waits for the full 4MB embed transfer.
  - Scalar (ACT) does Q/K evacuation during the QKV phase and *only* exp
    during attention — keeping its FIFO free of anything that could block
    the softmax stream.
  - k-bias is dropped entirely (softmax over keys is invariant to it).
  - Softmax normalization avoids DRAM round-trips: the denominator row is
    reshaped to [128,8] by an on-chip DMA, reciprocated wide on the DVE,
    reshaped back, then broadcast across 64 partitions by a K=1 matmul
    against a ones-row; a DVE multiply produces normalized heads.
  - Output projection is row-sharded Wo; partials are written in bf16 and
    summed on the host. All DMA triggers live on sync/vector/gpsimd rings.

Matmuls run in bf16 (host pre-rounds) with fp32 PSUM accumulation.
"""

import numpy as np

import concourse.tile as tile_mod
from concourse import bacc, mybir
from concourse.bass_utils import run_bass_kernel_spmd
from concourse.vector_clock import ScopedClock, VectorClock

F32 = mybir.dt.float32
BF16 = mybir.dt.bfloat16

S, D, H, DK = 2048, 1024, 16, 64
P = 128
NCORES = 8


def _patched_drain_and_barrier(self, tick_clock, wait_clock):
    """This container's walrus build caps CTRL-type instructions at one sem
    wait, but Tile's exit drain carries one wait per outstanding proc. Emit
    one Drain per outstanding proc instead, each with a single wait."""
    gc = tick_clock.global_clock
    vec = list(gc)
    for i, t in enumerate(vec):
        if t <= 0:
            continue
        pv = [0] * len(vec)
        pv[i] = t
        d = self.nc.sync.drain()
        wait_clock.add_sem_waits(d.ins, ScopedClock({None: VectorClock(pv)}))

    self.nc.all_engine_barrier()
    assert self.sems is not None
    popped = self.nc._tile_sem_poison_stack.pop()
    assert popped is self._sem_poison
    self.nc.clear_and_free_semaphores(list(self.sems.allocated().values()))
    self.nc.all_engine_barrier()


tile_mod.TileContext._drain_and_barrier = _patched_drain_and_barrier


def _build_nc():
    from contextlib import ExitStack

    tile = tile_mod
    nc = bacc.Bacc(None)

    et = nc.declare_dram_parameter("et", [D, S], BF16, isOutput=False)
    wqkv = nc.declare_dram_parameter("wqkv", [D, 6 * DK], BF16, isOutput=False)
    bq = nc.declare_dram_parameter("bq", [P, 1], F32, isOutput=False)
    bv = nc.declare_dram_parameter("bv", [P, 512], F32, isOutput=False)
    bo = nc.declare_dram_parameter("bo", [P, 8], F32, isOutput=False)
    wo = nc.declare_dram_parameter("wo", [P, D], BF16, isOutput=False)
    out = nc.declare_dram_parameter("out", [D, S], BF16, isOutput=True)

    et3 = et.rearrange("(po pi) s -> pi po s", pi=P)      # [128, 8, 2048]
    wqkv3 = wqkv.rearrange("(po pi) c -> pi po c", pi=P)  # [128, 8, 384]

    with tile.TileContext(nc) as tc, ExitStack() as ctx:
        consts = ctx.enter_context(tc.tile_pool(name="consts", bufs=1))
        qkv = ctx.enter_context(tc.tile_pool(name="qkv", bufs=1))
        utp = ctx.enter_context(tc.tile_pool(name="ut", bufs=4))
        headsp = ctx.enter_context(tc.tile_pool(name="heads", bufs=2))
        normp = ctx.enter_context(tc.tile_pool(name="norm", bufs=4))
        outp = ctx.enter_context(tc.tile_pool(name="outp", bufs=4))
        psum = ctx.enter_context(tc.tile_pool(name="psum", bufs=1, space="PSUM"))

        # ---- constants (small, on gpsimd ring so they never queue behind et)
        wqkv_sb = consts.tile([P, 8, 6 * DK], BF16)
        nc.gpsimd.dma_start(wqkv_sb[:], wqkv3[:])
        bq_sb = consts.tile([P, 1], F32)
        nc.gpsimd.dma_start(bq_sb[:], bq[:])
        bv_sb = consts.tile([P, 4, P], F32)
        nc.gpsimd.dma_start(bv_sb[:], bv.rearrange("p (a b) -> p a b", a=4))
        bo_c = consts.tile([P, 8], F32)
        nc.gpsimd.dma_start(bo_c[:], bo[:])
        wo_sb = consts.tile([P, D], BF16)
        nc.gpsimd.dma_start(wo_sb[:], wo[:])
        ones_sb = consts.tile([1, 64], F32)
        nc.vector.memset(ones_sb[:], 1.0)

        # ---- QKV phase: s-chunk-major pipeline --------------------------
        # et chunk c = all 1024 d-rows for s in [c*512, (c+1)*512); Q/K/V
        # matmuls for chunk c start as soon as it lands.
        et_sb = consts.tile([P, 8, S], BF16)
        qt_sb = qkv.tile([P, S], BF16)
        kt_sb = qkv.tile([P, S], BF16)
        vaug_sb = qkv.tile([P, 16, 130], BF16)
        nc.vector.memset(vaug_sb[:, :, 64:65], 1.0)
        nc.vector.memset(vaug_sb[:, :, 129:130], 1.0)

        for c in range(4):
            s0 = c * 512
            eng = nc.sync if c % 2 == 0 else nc.scalar
            eng.dma_start(et_sb[:, :, s0 : s0 + 512], et3[:, :, s0 : s0 + 512])

        # QKV-phase psum tiles rotate over the attention-phase slots
        # (st x2, av, op x2) so the pool fits in 8 banks.
        mm_rot = [("st", 2), ("st", 2), ("av", 1), ("op", 2), ("op", 2)]
        rot_i = 0

        def next_slot():
            nonlocal rot_i
            t = mm_rot[rot_i % 5]
            rot_i += 1
            return t

        for c in range(4):
            s0 = c * 512
            # Q^T / K^T for this chunk: psum [128, 512] accumulated over dc.
            for which, dst in ((0, qt_sb), (1, kt_sb)):
                tg, bfs = next_slot()
                ps = psum.tile([P, 512], F32, tag=tg, bufs=bfs, name=f"qk{c}{which}")
                for dc in range(8):
                    nc.tensor.matmul(
                        ps[:],
                        wqkv_sb[:, dc, which * 128 : which * 128 + 128],
                        et_sb[:, dc, s0 : s0 + 512],
                        start=(dc == 0),
                        stop=(dc == 7),
                    )
                if which == 0:
                    nc.scalar.activation(
                        dst[:, s0 : s0 + 512], ps[:],
                        mybir.ActivationFunctionType.Identity,
                        bias=bq_sb[:, 0:1],
                    )
                else:
                    nc.scalar.activation(
                        dst[:, s0 : s0 + 512], ps[:],
                        mybir.ActivationFunctionType.Identity,
                    )
            # V natural [t, v] for the 4 t-blocks of this chunk; one psum
            # bank holds all 4 side by side.
            tg, bfs = next_slot()
            psv = psum.tile([P, 4, P], F32, tag=tg, bufs=bfs, name=f"v{c}")
            for tl in range(4):
                t0 = s0 + tl * P
                for dc in range(8):
                    nc.tensor.matmul(
                        psv[:, tl, :],
                        et_sb[:, dc, t0 : t0 + P],
                        wqkv_sb[:, dc, 256:384],
                        start=(dc == 0),
                        stop=(dc == 7),
                        skip_group_check=True,
                    )
            tb0 = c * 4
            nc.vector.tensor_tensor(
                vaug_sb[:, tb0 : tb0 + 4, 0:64], psv[:, :, 0:64],
                bv_sb[:, :, 0:64], mybir.AluOpType.add,
            )
            nc.vector.tensor_tensor(
                vaug_sb[:, tb0 : tb0 + 4, 65:129], psv[:, :, 64:128],
                bv_sb[:, :, 64:128], mybir.AluOpType.add,
            )

        # ---- attention + row-sharded output projection ------------------
        out_dma_engs = [nc.sync, nc.gpsimd]

        def emit_outproj(sh, heads_sb):
            # out^T[c, s] = wo_rows.T @ heads^T (+ bo); evac on DVE (bf16),
            # DMA spread over the non-scalar rings.
            for blk in range(8):
                c0 = blk * P
                for ch in range(2):
                    s0 = ch * 512
                    ps = psum.tile(
                        [P, 512], F32, tag="op", bufs=2, name=f"op_{sh}_{blk}_{ch}"
                    )
                    nc.tensor.matmul(
                        ps[:],
                        wo_sb[:, c0 : c0 + P],
                        heads_sb[:, s0 : s0 + 512],
                        start=True,
                        stop=True,
                    )
                    ot = outp.tile([P, 512], BF16, tag="out")
                    nc.vector.tensor_scalar_add(ot[:], ps[:], bo_c[:, blk : blk + 1])
                    eng = out_dma_engs[(blk * 2 + ch) % 2]
                    eng.dma_start(
                        out[c0 : c0 + P, sh * 1024 + s0 : sh * 1024 + s0 + 512],
                        ot[:],
                    )

        for sh in range(2):
            h0 = sh * 1024
            heads_sb = headsp.tile([P, 1024], BF16, tag="heads", name=f"heads{sh}")
            for h in range(2):
                hp = h * 64
                av = psum.tile([65, 1024], F32, tag="av", bufs=1, name=f"av{sh}{h}")
                for tb in range(16):
                    t0 = tb * P
                    st = psum.tile(
                        [P, 1024], F32, tag="st", bufs=2, name=f"st{sh}{h}{tb}"
                    )
                    for n0 in (0, 512):
                        nc.tensor.matmul(
                            st[:, n0 : n0 + 512],
                            kt_sb[hp : hp + 64, t0 : t0 + P],
                            qt_sb[hp : hp + 64, h0 + n0 : h0 + n0 + 512],
                            start=True,
                            stop=True,
                        )
                    ut = utp.tile([P, 1024], BF16, tag="ut", bufs=4)
                    nc.scalar.activation(
                        ut[:], st[:], mybir.ActivationFunctionType.Exp, scale=0.125
                    )
                    for n0 in (0, 512):
                        nc.tensor.matmul(
                            av[:, n0 : n0 + 512],
                            vaug_sb[:, tb, h * 65 : h * 65 + 65],
                            ut[:, n0 : n0 + 512],
                            start=(tb == 0),
                            stop=(tb == 15),
                            skip_group_check=True,
                        )
                # Evacuate values + denominator (frees the av bank), then
                # build the reciprocal broadcast without touching DRAM:
                # row → [128,8] via on-chip DMA, wide reciprocal, back to a
                # row, K=1 matmul against ones broadcasts it to 64
                # partitions in PSUM, DVE multiply normalizes.
                unnorm_sb = headsp.tile(
                    [64, 1024], F32, tag="unnorm", name=f"un{sh}{h}"
                )
                nc.vector.tensor_copy(unnorm_sb[:], av[0:64, :])
                dsb = normp.tile([1, 1024], F32, tag="denom_sb", name=f"dsb{sh}{h}")
                nc.vector.tensor_copy(dsb[:], av[64:65, :])
                rsh = normp.tile([P, 8], F32, tag="rsh")
                nc.gpsimd.dma_start(rsh[:], dsb[:])
                nc.vector.reciprocal(rsh[:], rsh[:])
                rrow = normp.tile([1, 1024], F32, tag="rrow", name=f"rr{sh}{h}")
                nc.gpsimd.dma_start(rrow[:], rsh[:])
                for n0 in (0, 512):
                    rps = psum.tile(
                        [64, 512], F32, tag="op", bufs=2, name=f"rp{sh}{h}{n0}"
                    )
                    nc.tensor.matmul(
                        rps[:],
                        ones_sb[:],
                        rrow[0:1, n0 : n0 + 512],
                        start=True,
                        stop=True,
                    )
                    nc.vector.tensor_tensor(
                        heads_sb[hp : hp + 64, n0 : n0 + 512],
                        unnorm_sb[:, n0 : n0 + 512],
                        rps[:],
                        mybir.AluOpType.mult,
                    )
            emit_outproj(sh, heads_sb)

    nc.finalize()
    return nc


_NC_CACHE = None


def _get_nc():
    global _NC_CACHE
    if _NC_CACHE is None:
        _NC_CACHE = _build_nc()
    return _NC_CACHE


def _make_in_maps(embeddings, Wq, bq, Wk, bk, Wv, bv, Wo, bo):
    import ml_dtypes

    bf16 = np.dtype(ml_dtypes.bfloat16)
    et = np.ascontiguousarray(embeddings.T.astype(bf16))  # [1024, 2048]
    in_maps = []
    for c in range(NCORES):
        hs = [2 * c, 2 * c + 1]
        wqkv = np.concatenate(
            [Wq[hs[0]], Wq[hs[1]], Wk[hs[0]], Wk[hs[1]], Wv[hs[0]], Wv[hs[1]]],
            axis=1,
        ).astype(bf16)  # [1024, 384]
        bq_c = np.concatenate([bq[hs[0]], bq[hs[1]]]).astype(np.float32)[:, None]
        bv_row = np.concatenate([bv[hs[0]], bv[hs[1]]])  # [128]
        bv_c = np.ascontiguousarray(
            np.broadcast_to(np.tile(bv_row, 4)[None, :], (P, 512)),
            dtype=np.float32,
        )
        bo_eff = bo if c == 0 else np.zeros_like(bo)
        in_maps.append(
            {
                "et": et,
                "wqkv": np.ascontiguousarray(wqkv),
                "bq": np.ascontiguousarray(bq_c),
                "bv": bv_c,
                "bo": np.ascontiguousarray(bo_eff.reshape(8, P).T, dtype=np.float32),
                "wo": np.ascontiguousarray(Wo[c * P : (c + 1) * P].astype(bf16)),
            }
        )
    return in_maps


def kernel(embeddings, Wq, bq, Wk, bk, Wv, bv, Wo, bo, **run_kwargs):
    """Full-input / full-output MHA. Shards across 8 NeuronCores internally."""
    nc = _get_nc()
    in_maps = _make_in_maps(
        np.asarray(embeddings, np.float32),
        np.asarray(Wq, np.float32),
        np.asarray(bq, np.float32),
        np.asarray(Wk, np.float32),
        np.asarray(bk, np.float32),
        np.asarray(Wv, np.float32),
        np.asarray(bv, np.float32),
        np.asarray(Wo, np.float32),
        np.asarray(bo, np.float32),
    )
    res = run_bass_kernel_spmd(nc, in_maps, list(range(NCORES)), **run_kwargs)
    # Unshard the row-parallel output projection: sum the per-core bf16
    # partials in fp32, then undo the on-chip out^T layout.
    acc = res.results[0]["out"].astype(np.float32)
    for r_ in res.results[1:]:
        acc += r_["out"].astype(np.float32)
    return np.ascontiguousarray(acc.T)


if __name__ == "__main__":
    rng = np.random.default_rng(0)
    emb = rng.standard_normal((S, D), dtype=np.float32)
    mk = lambda *sh: (rng.standard_normal(sh, dtype=np.float32) * 0.02)
    o = kernel(
        embeddings=emb,
        Wq=mk(H, D, DK), bq=mk(H, DK),
        Wk=mk(H, D, DK), bk=mk(H, DK),
        Wv=mk(H, D, DK), bv=mk(H, DK),
        Wo=mk(H * DK, D), bo=mk(D),
    )
    print(o.shape, o.dtype)


# revision 15
# speedup vs baseline: 1.0607x; 1.0607x over previous
"""Trainium2 Bass kernel: 16-head MHA (S=2048, D=1024, Dk=Dv=64) on 8 NeuronCores.

Sharding: tensor-parallel over heads — 2 heads per core. Each core projects
Q/K/V for its 2 heads, computes scores transposed S^T[t, s] = K_h Q_h^T,
applies exp on the Scalar engine (scale 1/8 fused), and accumulates
heads^T = V_aug^T @ exp(S^T) with a ones-column in V so the softmax
denominator falls out of the same matmul (PSUM row 64).

Engine/schedule discipline (v3):
  - et streams in s-chunk-major; Q/K/V matmuls start as each 512-col chunk
    lands. Weights land first on the sync ring.
  - Scalar (ACT) does Q/K evacuation during the QKV phase and *only* exp
    during attention, so the softmax stream is never head-of-line blocked.
  - k-bias dropped (softmax over keys is invariant to it).
  - Engines execute in program order, so cross-phase work is software
    pipelined at EMISSION time: each (sh,h)'s reciprocal-broadcast matmuls,
    normalize multiplies and outproj chunks are emitted a few t-blocks INTO
    the next phase, and each phase's AV matmuls are deferred a few t-blocks
    so they never stall the PE FIFO while the previous av bank drains.
  - Softmax normalization has no DRAM round-trips: denominator row →
    [128,8] by on-chip DMA, wide DVE reciprocal, back to a row, broadcast
    across 64 partitions by a K=1 matmul against ones, DVE multiply.
  - Output partials are bf16, summed on the host (row-sharded Wo).

Matmuls run in bf16 (host pre-rounds) with fp32 PSUM accumulation.
"""

import numpy as np

import concourse.tile as tile_mod
from concourse import bacc, mybir
from concourse.bass_utils import run_bass_kernel_spmd
from concourse.vector_clock import ScopedClock, VectorClock

F32 = mybir.dt.float32
BF16 = mybir.dt.bfloat16

S, D, H, DK = 2048, 1024, 16, 64
P = 128
NCORES = 8


def _patched_drain_and_barrier(self, tick_clock, wait_clock):
    """This container's walrus build caps CTRL-type instructions at one sem
    wait, but Tile's exit drain carries one wait per outstanding proc. Emit
    one Drain per outstanding proc instead, each with a single wait."""
    gc = tick_clock.global_clock
    vec = list(gc)
    for i, t in enumerate(vec):
        if t <= 0:
            continue
        pv = [0] * len(vec)
        pv[i] = t
        d = self.nc.sync.drain()
        wait_clock.add_sem_waits(d.ins, ScopedClock({None: VectorClock(pv)}))

    self.nc.all_engine_barrier()
    assert self.sems is not None
    popped = self.nc._tile_sem_poison_stack.pop()
    assert popped is self._sem_poison
    self.nc.clear_and_free_semaphores(list(self.sems.allocated().values()))
    self.nc.all_engine_barrier()


tile_mod.TileContext._drain_and_barrier = _patched_drain_and_barrier


def _build_nc():
    from contextlib import ExitStack

    tile = tile_mod
    nc = bacc.Bacc(None)

    et = nc.declare_dram_parameter("et", [D, S], BF16, isOutput=False)
    wqkv = nc.declare_dram_parameter("wqkv", [D, 6 * DK], BF16, isOutput=False)
    bq = nc.declare_dram_parameter("bq", [P, 1], F32, isOutput=False)
    bv = nc.declare_dram_parameter("bv", [P, 512], F32, isOutput=False)
    bo = nc.declare_dram_parameter("bo", [P, 8], F32, isOutput=False)
    wo = nc.declare_dram_parameter("wo", [P, D], BF16, isOutput=False)
    out = nc.declare_dram_parameter("out", [D, S], BF16, isOutput=True)

    et3 = et.rearrange("(po pi) s -> pi po s", pi=P)      # [128, 8, 2048]
    wqkv3 = wqkv.rearrange("(po pi) c -> pi po c", pi=P)  # [128, 8, 384]

    with tile.TileContext(nc) as tc, ExitStack() as ctx:
        consts = ctx.enter_context(tc.tile_pool(name="consts", bufs=1))
        qkv = ctx.enter_context(tc.tile_pool(name="qkv", bufs=1))
        utp = ctx.enter_context(tc.tile_pool(name="ut", bufs=6))
        headsp = ctx.enter_context(tc.tile_pool(name="heads", bufs=2))
        normp = ctx.enter_context(tc.tile_pool(name="norm", bufs=4))
        outp = ctx.enter_context(tc.tile_pool(name="outp", bufs=4))
        psum = ctx.enter_context(tc.tile_pool(name="psum", bufs=1, space="PSUM"))

        # ---- constants: weights first on the sync ring, rest on scalar ----
        wqkv_sb = consts.tile([P, 8, 6 * DK], BF16)
        nc.sync.dma_start(wqkv_sb[:], wqkv3[:])
        bq_sb = consts.tile([P, 1], F32)
        nc.scalar.dma_start(bq_sb[:], bq[:])
        bv_sb = consts.tile([P, 4, P], F32)
        nc.scalar.dma_start(bv_sb[:], bv.rearrange("p (a b) -> p a b", a=4))
        bo_c = consts.tile([P, 8], F32)
        nc.scalar.dma_start(bo_c[:], bo[:])
        wo_sb = consts.tile([P, D], BF16)
        nc.scalar.dma_start(wo_sb[:], wo[:])
        ones_sb = consts.tile([1, 64], F32)
        nc.vector.memset(ones_sb[:], 1.0)

        # ---- QKV phase: s-chunk-major pipeline --------------------------
        et_sb = consts.tile([P, 8, S], BF16)
        qt_sb = qkv.tile([P, S], BF16)
        kt_sb = qkv.tile([P, S], BF16)
        vaug_sb = qkv.tile([P, 16, 130], BF16)
        nc.vector.memset(vaug_sb[:, :, 64:65], 1.0)
        nc.vector.memset(vaug_sb[:, :, 129:130], 1.0)

        for c in range(4):
            s0 = c * 512
            eng = nc.sync if c % 2 == 0 else nc.scalar
            eng.dma_start(et_sb[:, :, s0 : s0 + 512], et3[:, :, s0 : s0 + 512])

        # QKV-phase psum tiles rotate over the attention-phase slots
        # (st x2, av, op x2) so the pool fits in 8 banks.
        mm_rot = [("st", 2), ("st", 2), ("av", 1), ("op", 2), ("op", 2)]
        rot_i = 0

        def next_slot():
            nonlocal rot_i
            t = mm_rot[rot_i % 5]
            rot_i += 1
            return t

        for c in range(4):
            s0 = c * 512
            for which, dst in ((0, qt_sb), (1, kt_sb)):
                tg, bfs = next_slot()
                ps = psum.tile([P, 512], F32, tag=tg, bufs=bfs, name=f"qk{c}{which}")
                for dc in range(8):
                    nc.tensor.matmul(
                        ps[:],
                        wqkv_sb[:, dc, which * 128 : which * 128 + 128],
                        et_sb[:, dc, s0 : s0 + 512],
                        start=(dc == 0),
                        stop=(dc == 7),
                    )
                if which == 0:
                    nc.scalar.activation(
                        dst[:, s0 : s0 + 512], ps[:],
                        mybir.ActivationFunctionType.Identity,
                        bias=bq_sb[:, 0:1],
                    )
                else:
                    nc.scalar.activation(
                        dst[:, s0 : s0 + 512], ps[:],
                        mybir.ActivationFunctionType.Identity,
                    )
            # V natural [t, v] for the 4 t-blocks of this chunk.
            tg, bfs = next_slot()
            psv = psum.tile([P, 4, P], F32, tag=tg, bufs=bfs, name=f"v{c}")
            for tl in range(4):
                t0 = s0 + tl * P
                for dc in range(8):
                    nc.tensor.matmul(
                        psv[:, tl, :],
                        et_sb[:, dc, t0 : t0 + P],
                        wqkv_sb[:, dc, 256:384],
                        start=(dc == 0),
                        stop=(dc == 7),
                        skip_group_check=True,
                    )
            tb0 = c * 4
            nc.vector.tensor_tensor(
                vaug_sb[:, tb0 : tb0 + 4, 0:64], psv[:, :, 0:64],
                bv_sb[:, :, 0:64], mybir.AluOpType.add,
            )
            nc.vector.tensor_tensor(
                vaug_sb[:, tb0 : tb0 + 4, 65:129], psv[:, :, 64:128],
                bv_sb[:, :, 64:128], mybir.AluOpType.add,
            )

        # ---- attention, software-pipelined emission ---------------------
        # Phases run in order (sh, h) = (0,0),(0,1),(1,0),(1,1); each phase
        # has 16 t-block steps (~1.06us each at the exp-bound pace).
        # Engines execute their queues in program order, so all slow
        # dependency chains are EMITTED a few steps late — deep enough into
        # the next phase that their inputs are ready when the PE/DVE reach
        # them. Steady-state timing (steps after a phase's end E):
        #   av(tb15) due E+6, denominator chain E+7, reciprocal-broadcast
        #   matmuls + multiplies E+9/E+10 (frees the av bank), next phase's
        #   first av matmuls E+10, outproj chunks E+11..E+26.
        AV_LAG = 7          # av matmuls trail the st/exp stream
        FIRST_FREE = 10     # first av matmuls of a phase wait out the
                            # previous phase's normalize chain (av bufs=1)
        pending = []        # (due_step, seq, fn)
        seq_n = 0
        step = 0

        def push(delay, fn):
            nonlocal seq_n
            pending.append((step + delay, seq_n, fn))
            seq_n += 1

        def drain_due():
            pending.sort()
            while pending and pending[0][0] <= step:
                _, _, fn = pending.pop(0)
                fn()

        def emit_outproj_chunk(sh, heads_sb, blk, ch):
            c0 = blk * P
            s0 = ch * 512
            ps = psum.tile(
                [P, 512], F32, tag="op", bufs=2, name=f"op_{sh}_{blk}_{ch}"
            )
            nc.tensor.matmul(
                ps[:],
                wo_sb[:, c0 : c0 + P],
                heads_sb[:, s0 : s0 + 512],
                start=True,
                stop=True,
            )
            ot = outp.tile([P, 512], BF16, tag="out")
            nc.vector.tensor_scalar_add(ot[:], ps[:], bo_c[:, blk : blk + 1])
            nc.sync.dma_start(
                out[c0 : c0 + P, sh * 1024 + s0 : sh * 1024 + s0 + 512],
                ot[:],
            )

        def queue_normalize_and_outproj(sh, h, av, heads_sb, last):
            hp = h * 64

            def chain_head():
                # values + denominator row out of PSUM (frees the av bank),
                # then denominator -> [128,8] -> wide reciprocal -> row.
                dsb = normp.tile(
                    [1, 1024], F32, tag="denom_sb", name=f"dsb{sh}{h}"
                )
                nc.vector.tensor_copy(dsb[:], av[64:65, :])
                nc.vector.tensor_copy(chain_head.unnorm[:], av[0:64, :])
                rsh = normp.tile([P, 8], F32, tag="rsh")
                nc.gpsimd.dma_start(rsh[:], dsb[:])
                nc.vector.reciprocal(rsh[:], rsh[:])
                nc.gpsimd.dma_start(chain_head.rrow[:], rsh[:])

            chain_head.rrow = normp.tile(
                [1, 1024], F32, tag="rrow", name=f"rr{sh}{h}"
            )
            chain_head.unnorm = headsp.tile(
                [64, 1024], F32, tag="unnorm", name=f"un{sh}{h}"
            )

            def mk_norm(n0):
                def fn():
                    rps = psum.tile(
                        [64, 512], F32, tag="op", bufs=2, name=f"rp{sh}{h}{n0}"
                    )
                    nc.tensor.matmul(
                        rps[:],
                        ones_sb[:],
                        chain_head.rrow[0:1, n0 : n0 + 512],
                        start=True,
                        stop=True,
                    )
                    nc.vector.tensor_tensor(
                        heads_sb[hp : hp + 64, n0 : n0 + 512],
                        chain_head.unnorm[:, n0 : n0 + 512],
                        rps[:],
                        mybir.AluOpType.mult,
                    )
                return fn

            if last:
                push(0, chain_head)
                push(0, mk_norm(0))
                push(0, mk_norm(512))
            else:
                push(AV_LAG, chain_head)
                push(AV_LAG + 2, mk_norm(0))
                push(AV_LAG + 3, mk_norm(512))
            if h == 1:
                for i, (blk, ch) in enumerate(
                    [(b, c) for b in range(8) for c in range(2)]
                ):
                    push(
                        0 if last else (AV_LAG + 4 + i),
                        (lambda b=blk, c=ch: emit_outproj_chunk(sh, heads_sb, b, c)),
                    )

        heads_tiles = {}
        for sh in range(2):
            heads_tiles[sh] = headsp.tile(
                [P, 1024], BF16, tag="heads", name=f"heads{sh}"
            )

        for sh in range(2):
            h0 = sh * 1024
            for h in range(2):
                hp = h * 64
                first = sh == 0 and h == 0
                last = sh == 1 and h == 1
                av = psum.tile([65, 1024], F32, tag="av", bufs=1, name=f"av{sh}{h}")
                for tb in range(16):
                    t0 = tb * P
                    st = psum.tile(
                        [P, 1024], F32, tag="st", bufs=2, name=f"st{sh}{h}{tb}"
                    )
                    for n0 in (0, 512):
                        nc.tensor.matmul(
                            st[:, n0 : n0 + 512],
                            kt_sb[hp : hp + 64, t0 : t0 + P],
                            qt_sb[hp : hp + 64, h0 + n0 : h0 + n0 + 512],
                            start=True,
                            stop=True,
                        )
                    ut = utp.tile([P, 1024], BF16, tag="ut", bufs=12)
                    nc.scalar.activation(
                        ut[:], st[:], mybir.ActivationFunctionType.Exp, scale=0.125
                    )

                    def mk_av(ut=ut, tb=tb, av=av, h=h):
                        def fn():
                            for n0 in (0, 512):
                                nc.tensor.matmul(
                                    av[:, n0 : n0 + 512],
                                    vaug_sb[:, tb, h * 65 : h * 65 + 65],
                                    ut[:, n0 : n0 + 512],
                                    start=(tb == 0),
                                    stop=(tb == 15),
                                    skip_group_check=True,
                                )
                        return fn

                    lag = AV_LAG if first else max(AV_LAG, FIRST_FREE - tb)
                    push(lag, mk_av())
                    step += 1
                    drain_due()
                if last:
                    # flush remaining av matmuls, then the tail chain.
                    step += AV_LAG + 1
                    drain_due()
                queue_normalize_and_outproj(sh, h, av, heads_tiles[sh], last)
                if last:
                    step += 64
                    drain_due()

    nc.finalize()
    return nc


_NC_CACHE = None


def _get_nc():
    global _NC_CACHE
    if _NC_CACHE is None:
        _NC_CACHE = _build_nc()
    return _NC_CACHE


def _make_in_maps(embeddings, Wq, bq, Wk, bk, Wv, bv, Wo, bo):
    import ml_dtypes

    bf16 = np.dtype(ml_dtypes.bfloat16)
    et = np.ascontiguousarray(embeddings.T.astype(bf16))  # [1024, 2048]
    in_maps = []
    for c in range(NCORES):
        hs = [2 * c, 2 * c + 1]
        wqkv = np.concatenate(
            [Wq[hs[0]], Wq[hs[1]], Wk[hs[0]], Wk[hs[1]], Wv[hs[0]], Wv[hs[1]]],
            axis=1,
        ).astype(bf16)  # [1024, 384]
        bq_c = np.concatenate([bq[hs[0]], bq[hs[1]]]).astype(np.float32)[:, None]
        bv_row = np.concatenate([bv[hs[0]], bv[hs[1]]])  # [128]
        bv_c = np.ascontiguousarray(
            np.broadcast_to(np.tile(bv_row, 4)[None, :], (P, 512)),
            dtype=np.float32,
        )
        bo_eff = bo if c == 0 else np.zeros_like(bo)
        in_maps.append(
            {
                "et": et,
                "wqkv": np.ascontiguousarray(wqkv),
                "bq": np.ascontiguousarray(bq_c),
                "bv": bv_c,
                "bo": np.ascontiguousarray(bo_eff.reshape(8, P).T, dtype=np.float32),
                "wo": np.ascontiguousarray(Wo[c * P : (c + 1) * P].astype(bf16)),
            }
        )
    return in_maps


def kernel(embeddings, Wq, bq, Wk, bk, Wv, bv, Wo, bo, **run_kwargs):
    """Full-input / full-output MHA. Shards across 8 NeuronCores internally."""
    nc = _get_nc()
    in_maps = _make_in_maps(
        np.asarray(embeddings, np.float32),
        np.asarray(Wq, np.float32),
        np.asarray(bq, np.float32),
        np.asarray(Wk, np.float32),
        np.asarray(bk, np.float32),
        np.asarray(Wv, np.float32),
        np.asarray(bv, np.float32),
        np.asarray(Wo, np.float32),
        np.asarray(bo, np.float32),
    )
    res = run_bass_kernel_spmd(nc, in_maps, list(range(NCORES)), **run_kwargs)
    # Unshard the row-parallel output projection: sum the per-core bf16
    # partials in fp32, then undo the on-chip out^T layout.
    acc = res.results[0]["out"].astype(np.float32)
    for r_ in res.results[1:]:
        acc += r_["out"].astype(np.float32)
    return np.ascontiguousarray(acc.T)


if __name__ == "__main__":
    rng = np.random.default_rng(0)
    emb = rng.standard_normal((S, D), dtype=np.float32)
    mk = lambda *sh: (rng.standard_normal(sh, dtype=np.float32) * 0.02)
    o = kernel(
        embeddings=emb,
        Wq=mk(H, D, DK), bq=mk(H, DK),
        Wk=mk(H, D, DK), bk=mk(H, DK),
        Wv=mk(H, D, DK), bv=mk(H, DK),
        Wo=mk(H * DK, D), bo=mk(D),
    )
    print(o.shape, o.dtype)
